# revision 1
# baseline (speedup 1.0000x reference)
"""DiffPool-style GNN message passing on 8 trn2 NeuronCores.

Reference computation (N=4096, F_IN=512, F_OUT=4096):
    h = relu(x @ W1 + b1)            [N, F_OUT]
    s = relu(x @ W2 + b2)            [N, F_OUT]
    a = exp(h @ W3 + b3) * adj       [N, N]
    a = a / rowsum(a)
    out = a @ s                      [N, F_OUT]

Sharding: 1D row-parallel over 8 cores (512 rows each). Each core computes
its row-shard of h, s, a. The full s (needed as the RHS of a @ s) is
assembled with an on-chip AllGather of the per-core s shards, overlapped
with the big h @ W3 GEMM.

Layout trick: everything that would need an on-chip transpose is fed
pre-transposed from the host (xT, adjT), and h / e are produced directly in
transposed layout so both big GEMMs take them as the stationary operand
with zero on-chip transposes.

Precision: the h @ W3 GEMM runs in fp8e4 DoubleRow (2x PE throughput; W3 is
host-prescaled by 64 into fp8's normal range and the scale is undone inside
the fused exp) — row normalization cancels logit-scale errors, so this is
accuracy-neutral vs bf16. All other matmuls are bf16 with f32 PSUM
accumulation. Row normalization is applied by scaling the output rows with
1/rowsum (mathematically identical). The final a @ s GEMM can also run in
fp8 (fp8_d=True, ~1.3x faster overall) at ~6x the relative error — off by
default.
"""

import numpy as np
import ml_dtypes

import concourse.bass as bass
import concourse.mybir as mybir
import concourse.tile as tile
from concourse import bacc
from concourse.bass import ts
from concourse.bass_utils import run_bass_kernel_spmd

BF = mybir.dt.bfloat16
F32 = mybir.dt.float32
F8 = mybir.dt.float8e4
W3_SCALE = 64.0  # W3 values (~1e-3 std) are rescaled into fp8e4's normal range
W1_SCALE = 16.0  # W1 likewise; hT then holds 16*h, undone in the exp scale
W2_SCALE = 16.0  # W2 likewise; psum holds 16*(x@W2+b2), undone in the relu scale

N = 4096
F_IN = 512
F_OUT = 4096
NCORES = 8
R = N // NCORES  # 512 rows per core

AFT = mybir.ActivationFunctionType
ALU = mybir.AluOpType


def build_nc(loop_reps: int | None = None, with_b2: bool = True,
             parts: str = "bacd", fp8_d: bool = False) -> bass.Bass:
    """loop_reps: timing-only variant — wraps the compute stages in a dynamic
    For_i loop (collective hoisted out, since collectives can't sit inside
    control flow) so per-iteration time can be measured past the ~100ms
    axon dispatch jitter."""
    nc = bacc.Bacc("TRN2", target_bir_lowering=False, debug=False, num_devices=NCORES)

    xT_d = nc.dram_tensor("xT", [F_IN, R], BF, kind="ExternalInput")
    xT8_d = nc.dram_tensor("xT8", [F_IN, R], F8, kind="ExternalInput")
    w1_d = nc.dram_tensor("w1", [F_IN, F_OUT], F8, kind="ExternalInput")
    w2_d = nc.dram_tensor("w2", [F_IN, F_OUT], BF, kind="ExternalInput")
    w3_d = nc.dram_tensor("w3", [F_OUT, N], F8, kind="ExternalInput")
    adjT_d = nc.dram_tensor("adjT", [N, R], BF, kind="ExternalInput")
    b1_d = nc.dram_tensor("b1", [F_OUT], F32, kind="ExternalInput")
    b2_d = nc.dram_tensor("b2", [F_OUT], F32, kind="ExternalInput")
    b3_d = nc.dram_tensor("b3", [N], F32, kind="ExternalInput")
    out_d = nc.dram_tensor("out", [R, F_OUT], F32, kind="ExternalOutput")

    # Partition-tiled views (partition dim = 128 always first).
    xT_v = xT_d.rearrange("(kt p) r -> p kt r", p=128)      # [128, 4, 512]
    xT8_v = xT8_d.rearrange("(kt p) r -> p kt r", p=128)    # [128, 4, 512]
    w1_v = w1_d.rearrange("(kt p) f -> p kt f", p=128)      # [128, 4, 4096]
    w2_v = w2_d.rearrange("(kt p) f -> p kt f", p=128)      # [128, 4, 4096]
    w3_v = w3_d.rearrange("(kt p) c -> p kt c", p=128)      # [128, 32, 4096]
    adjT_v = adjT_d.rearrange("(ct p) r -> p ct r", p=128)  # [128, 32, 512]
    b1_v = b1_d.rearrange("(t p) -> p t", p=128)            # [128, 32]
    b3_v = b3_d.rearrange("(t p) -> p t", p=128)            # [128, 32]
    out_v = out_d.rearrange("(rt p) c -> p rt c", p=128)    # [128, 4, 4096]

    KT_IN = F_IN // 128    # 4 k-tiles for the x-side GEMMs
    KT_BIG = F_OUT // 128  # 32 k-tiles for the big GEMMs
    NB = F_OUT // 512      # 8 column blocks of 512
    RT = R // 128          # 4 row tiles per shard
    KQ = 8                 # k-tiles per streamed quarter of W3 / s

    with tile.TileContext(nc) as tc:
        with (
            tc.tile_pool(name="const", bufs=1) as constp,
            tc.tile_pool(name="hpool", bufs=1) as hp,
            tc.tile_pool(name="epool", bufs=1) as ep,
            tc.tile_pool(name="w1p", bufs=2) as w1p,
            tc.tile_pool(name="w2p", bufs=2) as w2p,
            tc.tile_pool(name="sB", bufs=4) as sBp,
            tc.tile_pool(name="w3p", bufs=6) as w3p,
            tc.tile_pool(name="adjp", bufs=3) as adjp,
            tc.tile_pool(name="expp", bufs=4) as expp,
            tc.tile_pool(name="sDp", bufs=6) as sDp,
            tc.tile_pool(name="outp", bufs=4) as outp,
            tc.tile_pool(name="psum_big", bufs=7, space="PSUM") as psumb,
            tc.tile_pool(name="psum_d", bufs=1, space="PSUM") as psumd,
            tc.tile_pool(name="dram", bufs=1, space="DRAM") as dramp,
        ):
            # ---- persistent constants / small tensors ----
            xT_sb = constp.tile([128, KT_IN, R], BF)
            nc.sync.dma_start(xT_sb[:], xT_v[:])
            xT8_sb = constp.tile([128, KT_IN, R], F8)
            nc.sync.dma_start(xT8_sb[:], xT8_v[:])
            b1_sb = constp.tile([128, KT_BIG], F32)
            nc.sync.dma_start(b1_sb[:], b1_v[:])
            b3_sb = constp.tile([128, KT_BIG], F32)
            nc.sync.dma_start(b3_sb[:], b3_v[:])
            if with_b2:
                b2row_f = constp.tile([1, F_OUT], F32)
                nc.sync.dma_start(b2row_f[:], b2_d[None, :])
                b2row = constp.tile([1, F_OUT], BF)
                nc.vector.tensor_copy(b2row[:], b2row_f[:])
                ones_row = constp.tile([1, 128], BF)
                nc.vector.memset(ones_row[:], 1.0)
            ones_col = constp.tile([128, 1], F32)
            nc.vector.memset(ones_col[:], 1.0)
            dAcc = constp.tile([128, R], F32)
            d_row = constp.tile([1, R], F32)
            dT_sb = constp.tile([128, RT], F32)
            rd_sb = constp.tile([128, RT], F32)

            SD = F8 if fp8_d else BF
            hT_sb = hp.tile([128, KT_BIG, R], F8)   # h transposed  [F_OUT, R]
            eT_sb = ep.tile([128, KT_BIG, R], SD)   # e transposed  [N, R]

            s_in_dram = dramp.tile([R, F_OUT], SD)
            s_out_dram = dramp.tile([N, F_OUT], SD)
            d_dram = dramp.tile([R], F32)
            s_in_v = s_in_dram.rearrange("(rt p) c -> p rt c", p=128)
            s_out_v = s_out_dram.rearrange("(kt p) c -> p kt c", p=128)

            def stage_b_block(cb):
                for cb in [cb]:
                    w2_sb = w2p.tile([128, KT_IN, 512], BF, name="w2_sb")
                    nc.sync.dma_start(w2_sb[:], w2_v[:, :, ts(cb, 512)])
                    for rt in range(RT):
                        psB = psumb.tile([128, 512], F32, name="ps", tag="ps")
                        for kt in range(KT_IN):
                            nc.tensor.matmul(
                                psB[:],
                                xT_sb[:, kt, ts(rt, 128)],
                                w2_sb[:, kt, :],
                                start=(kt == 0),
                                stop=(not with_b2 and kt == KT_IN - 1),
                            )
                        if with_b2:
                            # broadcast b2 over rows via K=1 matmul: psum += 1*b2
                            nc.tensor.matmul(
                                psB[:],
                                ones_row[:],
                                b2row[:, ts(cb, 512)],
                                start=False,
                                stop=True,
                            )
                        s_sb = sBp.tile([128, 512], SD, name="s_sb")
                        nc.scalar.activation(s_sb[:], psB[:], AFT.Relu)
                        nc.sync.dma_start(s_in_v[:, rt, ts(cb, 512)], s_sb[:])

            def stage_b():
                # s_i = relu(x_i @ W2 + b2), row-major -> DRAM bounce
                for cb in range(NB):
                    stage_b_block(cb)

            def stage_ba():
                # interleave B and A column-blocks: alternating independent
                # psum groups keep PE fed across group boundaries
                for blk in range(NB):
                    stage_b_block(blk)
                    stage_a_block(blk)

            def all_gather():
                nc.gpsimd.collective_compute(
                    "AllGather",
                    ALU.bypass,
                    replica_groups=[list(range(NCORES))],
                    ins=[s_in_dram[:]],
                    outs=[s_out_dram[:]],
                )

            def stage_a_block(fg):
                # hT = relu(x_i @ (16*W1) + 16*b1)^T = 16*h^T; fp8 DoubleRow
                w1_sb = w1p.tile([128, KT_IN, 512], F8, name="w1_sb")
                nc.sync.dma_start(w1_sb[:], w1_v[:, :, ts(fg, 512)])
                for fw in range(4):
                    ft = fg * 4 + fw
                    psA = psumb.tile([128, R], F32, name="ps", tag="ps")
                    for u in range(KT_IN // 2):
                        nc.tensor.matmul(
                            psA[:],
                            w1_sb[:, 2 * u : 2 * u + 2, ts(fw, 128)],
                            xT8_sb[:, 2 * u : 2 * u + 2, :],
                            start=(u == 0),
                            stop=(u == KT_IN // 2 - 1),
                            perf_mode=mybir.MatmulPerfMode.DoubleRow,
                        )
                    nc.scalar.activation(
                        hT_sb[:, ft, :], psA[:], AFT.Relu,
                        bias=b1_sb[:, ft : ft + 1],
                    )

            def stage_a():
                for fg in range(NB):
                    stage_a_block(fg)

            def stage_c():
                # eT = (exp(h @ W3 + b3) * adj)^T  [N, R]; dAcc accumulation
                for cb in range(NB):
                    w3_q = []
                    for q in range(KT_BIG // KQ):
                        w3_sb = w3p.tile([128, KQ, 512], F8, name="w3_sb")
                        nc.sync.dma_start(
                            w3_sb[:], w3_v[:, ts(q, KQ), ts(cb, 512)]
                        )
                        w3_q.append(w3_sb)
                    adj_sb = adjp.tile([128, 4, R], BF, name="adj_sb")
                    nc.sync.dma_start(adj_sb[:], adjT_v[:, ts(cb, 4), :])
                    for cw in range(4):
                        ct = cb * 4 + cw
                        psC = psumb.tile([128, R], F32, name="ps", tag="ps")
                        # fp8 DoubleRow: each matmul contracts a PAIR of
                        # k-tiles (middle AP dim = the 2 interleaved rows)
                        NP = KT_BIG // 2
                        for u in range(NP):
                            q, r = (2 * u) // KQ, (2 * u) % KQ
                            nc.tensor.matmul(
                                psC[:],
                                w3_q[q][:, r : r + 2, ts(cw, 128)],
                                hT_sb[:, 2 * u : 2 * u + 2, :],
                                start=(u == 0),
                                stop=(u == NP - 1),
                                perf_mode=mybir.MatmulPerfMode.DoubleRow,
                            )
                        ex_sb = expp.tile([128, R], BF, name="ex_sb")
                        nc.scalar.activation(
                            ex_sb[:], psC[:], AFT.Exp,
                            bias=b3_sb[:, ct : ct + 1],
                            scale=1.0 / (W3_SCALE * W1_SCALE),
                        )
                        nc.vector.tensor_tensor(
                            eT_sb[:, ct, :], ex_sb[:], adj_sb[:, cw, :], op=ALU.mult
                        )
                        if ct == 0:
                            nc.vector.tensor_copy(dAcc[:], eT_sb[:, ct, :])
                        else:
                            nc.vector.tensor_tensor(
                                dAcc[:], dAcc[:], eT_sb[:, ct, :], op=ALU.add
                            )

                # rowsums: reduce dAcc over partitions with a ones matmul,
                # round-trip through DRAM to relayout [1,512] -> [128,4]
                psD1 = psumd.tile([1, R], F32, name="psD1", tag="psD1")
                nc.tensor.matmul(psD1[:], ones_col[:], dAcc[:], start=True, stop=True)
                nc.scalar.copy(d_row[:], psD1[:])
                nc.sync.dma_start(d_dram.rearrange("(a r) -> a r", a=1), d_row[:])
                nc.sync.dma_start(dT_sb[:], d_dram.rearrange("(t p) -> p t", p=128))
                nc.vector.reciprocal(rd_sb[:], dT_sb[:])

            def stage_d():
                # out_i = diag(1/d) (e_i @ s)  [R, F_OUT]
                for cb in range(NB):
                    s_q = []
                    for q in range(KT_BIG // KQ):
                        sD_sb = sDp.tile([128, KQ, 512], SD, name="sD_sb")
                        nc.sync.dma_start(
                            sD_sb[:], s_out_v[:, ts(q, KQ), ts(cb, 512)]
                        )
                        s_q.append(sD_sb)
                    for rt in range(RT):
                        psE = psumb.tile([128, 512], F32, name="ps", tag="ps")
                        if fp8_d:
                            NP = KT_BIG // 2
                            for u in range(NP):
                                q, r = (2 * u) // KQ, (2 * u) % KQ
                                nc.tensor.matmul(
                                    psE[:],
                                    eT_sb[:, 2 * u : 2 * u + 2, ts(rt, 128)],
                                    s_q[q][:, r : r + 2, :],
                                    start=(u == 0),
                                    stop=(u == NP - 1),
                                    perf_mode=mybir.MatmulPerfMode.DoubleRow,
                                )
                        else:
                            for kt in range(KT_BIG):
                                nc.tensor.matmul(
                                    psE[:],
                                    eT_sb[:, kt, ts(rt, 128)],
                                    s_q[kt // KQ][:, kt % KQ, :],
                                    start=(kt == 0),
                                    stop=(kt == KT_BIG - 1),
                                )
                        ob = outp.tile([128, 512], F32, name="ob")
                        nc.vector.tensor_scalar_mul(
                            ob[:], psE[:], rd_sb[:, rt : rt + 1]
                        )
                        nc.sync.dma_start(out_v[:, rt, ts(cb, 512)], ob[:])

            if loop_reps is None:
                stage_ba()
                all_gather()
                stage_c()
                stage_d()
            else:
                stage_b()
                all_gather()
                stage_a()
                stage_c()  # so eT/rd are valid even if the loop omits stages
                with tc.For_i(0, loop_reps, 1):
                    if "b" in parts and "a" in parts:
                        stage_ba()
                    elif "b" in parts:
                        stage_b()
                    elif "a" in parts:
                        stage_a()
                    if "c" in parts:
                        stage_c()
                    if "d" in parts:
                        stage_d()

    nc.compile()
    return nc


def make_in_maps(x, adj, W1, b1, W2, b2, W3, b3):
    bf = ml_dtypes.bfloat16
    xT = np.ascontiguousarray(x.T).astype(bf)        # [F_IN, N]
    adjT = np.ascontiguousarray(adj.T).astype(bf)    # [N, N] (cols x rows)
    f8 = ml_dtypes.float8_e4m3
    w3b = np.clip(np.ascontiguousarray(W3) * W3_SCALE, -240, 240).astype(f8)
    w1b = np.clip(np.ascontiguousarray(W1) * W1_SCALE, -240, 240).astype(f8)
    w2b = np.ascontiguousarray(W2).astype(bf)
    xT8 = np.clip(xT.astype(np.float32), -240, 240).astype(f8)
    b1f = np.ascontiguousarray(b1).astype(np.float32) * np.float32(W1_SCALE)
    b2f = np.ascontiguousarray(b2).astype(np.float32)
    b3f = np.ascontiguousarray(b3).astype(np.float32)
    in_maps = []
    for i in range(NCORES):
        sl = slice(i * R, (i + 1) * R)
        in_maps.append(
            {
                "xT": np.ascontiguousarray(xT[:, sl]),
                "xT8": np.ascontiguousarray(xT8[:, sl]),
                "adjT": np.ascontiguousarray(adjT[:, sl]),
                "w1": w1b,
                "w2": w2b,
                "w3": w3b,
                "b1": b1f,
                "b2": b2f,
                "b3": b3f,
            }
        )
    return in_maps


def run(x, adj, W1, b1, W2, b2, W3, b3, trace=False, fp8_d=False):
    nc = build_nc(with_b2=bool(np.any(np.asarray(b2))), fp8_d=fp8_d)
    in_maps = make_in_maps(x, adj, W1, b1, W2, b2, W3, b3)
    res = run_bass_kernel_spmd(nc, in_maps, core_ids=list(range(NCORES)), trace=trace)
    out = np.concatenate([res.results[i]["out"] for i in range(NCORES)], axis=0)
    return out.astype(np.float32), res


def kernel(x, adj, W1, b1, W2, b2, W3, b3):
    args = [np.asarray(a) for a in (x, adj, W1, b1, W2, b2, W3, b3)]
    out, _ = run(*args, trace=False)
    return out



# revision 6
# speedup vs baseline: 4.4937x; 4.4937x over previous
"""DiffPool-style GNN message passing on 8 trn2 NeuronCores.

Reference computation (N=4096, F_IN=512, F_OUT=4096):
    h = relu(x @ W1 + b1)            [N, F_OUT]
    s = relu(x @ W2 + b2)            [N, F_OUT]
    a = exp(h @ W3 + b3) * adj       [N, N]
    a = a / rowsum(a)
    out = a @ s                      [N, F_OUT]

Sharding: 1D row-parallel over 8 cores (512 rows each). Each core computes
its row-shard of h, s, a. The full s (needed as the RHS of a @ s) is
assembled with an on-chip AllGather of the per-core s shards, overlapped
with the big h @ W3 GEMM.

Layout trick: everything that would need an on-chip transpose is fed
pre-transposed from the host (xT, adjT), and h / e are produced directly in
transposed layout so both big GEMMs take them as the stationary operand
with zero on-chip transposes.

Precision: the h @ W3 GEMM runs in fp8e4 DoubleRow (2x PE throughput; W3 is
host-prescaled by 64 into fp8's normal range and the scale is undone inside
the fused exp) — row normalization cancels logit-scale errors, so this is
accuracy-neutral vs bf16. All other matmuls are bf16 with f32 PSUM
accumulation. Row normalization is applied by scaling the output rows with
1/rowsum (mathematically identical). The final a @ s GEMM can also run in
fp8 (fp8_d=True, ~1.3x faster overall) at ~6x the relative error — off by
default.
"""

import numpy as np
import ml_dtypes

import concourse.bass as bass
import concourse.mybir as mybir
import concourse.tile as tile
from concourse import bacc
from concourse.bass import ts
from concourse.bass_utils import run_bass_kernel_spmd

BF = mybir.dt.bfloat16
F32 = mybir.dt.float32
F8 = mybir.dt.float8e4
W3_SCALE = 64.0  # W3 values (~1e-3 std) are rescaled into fp8e4's normal range
W1_SCALE = 16.0  # W1 likewise; hT then holds 16*h, undone in the exp scale
W2_SCALE = 16.0  # W2 likewise; psum holds 16*(x@W2+b2), undone in the relu scale

N = 4096
F_IN = 512
F_OUT = 4096
NCORES = 8
R = N // NCORES  # 512 rows per core

AFT = mybir.ActivationFunctionType
ALU = mybir.AluOpType


def build_nc(loop_reps: int | None = None, with_b2: bool = True,
             parts: str = "bacd", fp8_d: bool = True) -> bass.Bass:
    """loop_reps: timing-only variant — wraps the compute stages in a dynamic
    For_i loop (collective hoisted out, since collectives can't sit inside
    control flow) so per-iteration time can be measured past the ~100ms
    axon dispatch jitter."""
    nc = bacc.Bacc("TRN2", target_bir_lowering=False, debug=False, num_devices=NCORES)

    xT_d = nc.dram_tensor("xT", [F_IN, R], BF, kind="ExternalInput")
    xT8_d = nc.dram_tensor("xT8", [F_IN, R], F8, kind="ExternalInput")
    w1_d = nc.dram_tensor("w1", [F_IN, F_OUT], F8, kind="ExternalInput")
    w2_d = nc.dram_tensor("w2", [F_IN, F_OUT], F8, kind="ExternalInput")
    w3_d = nc.dram_tensor("w3", [F_OUT, N], F8, kind="ExternalInput")
    adjT_d = nc.dram_tensor("adjT", [N, R], BF, kind="ExternalInput")
    b1_d = nc.dram_tensor("b1", [F_OUT], F32, kind="ExternalInput")
    b2_d = nc.dram_tensor("b2", [F_OUT], F32, kind="ExternalInput")
    b3_d = nc.dram_tensor("b3", [N], F32, kind="ExternalInput")
    out_d = nc.dram_tensor("out", [R, F_OUT], F32, kind="ExternalOutput")

    # Partition-tiled views (partition dim = 128 always first).
    xT_v = xT_d.rearrange("(kt p) r -> p kt r", p=128)      # [128, 4, 512]
    xT8_v = xT8_d.rearrange("(kt p) r -> p kt r", p=128)    # [128, 4, 512]
    w1_v = w1_d.rearrange("(kt p) f -> p kt f", p=128)      # [128, 4, 4096]
    w2_v = w2_d.rearrange("(kt p) f -> p kt f", p=128)      # [128, 4, 4096]
    w3_v = w3_d.rearrange("(kt p) c -> p kt c", p=128)      # [128, 32, 4096]
    adjT_v = adjT_d.rearrange("(ct p) r -> p ct r", p=128)  # [128, 32, 512]
    b1_v = b1_d.rearrange("(t p) -> p t", p=128)            # [128, 32]
    b3_v = b3_d.rearrange("(t p) -> p t", p=128)            # [128, 32]
    out_v = out_d.rearrange("(rt p) c -> p rt c", p=128)    # [128, 4, 4096]

    KT_IN = F_IN // 128    # 4 k-tiles for the x-side GEMMs
    KT_BIG = F_OUT // 128  # 32 k-tiles for the big GEMMs
    NB = F_OUT // 512      # 8 column blocks of 512
    RT = R // 128          # 4 row tiles per shard
    KQ = 8                 # k-tiles per streamed quarter of W3 / s

    with tile.TileContext(nc) as tc:
        with (
            tc.tile_pool(name="const", bufs=1) as constp,
            tc.tile_pool(name="hpool", bufs=1) as hp,
            tc.tile_pool(name="epool", bufs=1) as ep,
            tc.tile_pool(name="w1p", bufs=2) as w1p,
            tc.tile_pool(name="w2p", bufs=2) as w2p,
            tc.tile_pool(name="sB", bufs=4) as sBp,
            tc.tile_pool(name="w3p", bufs=6) as w3p,
            tc.tile_pool(name="adjp", bufs=3) as adjp,
            tc.tile_pool(name="expp", bufs=4) as expp,
            tc.tile_pool(name="sDp", bufs=6) as sDp,
            tc.tile_pool(name="outp", bufs=4) as outp,
            tc.tile_pool(name="psum_big", bufs=7, space="PSUM") as psumb,
            tc.tile_pool(name="psum_d", bufs=1, space="PSUM") as psumd,
            tc.tile_pool(name="dram", bufs=1, space="DRAM") as dramp,
        ):
            # ---- persistent constants / small tensors ----
            xT_sb = constp.tile([128, KT_IN, R], BF)
            nc.sync.dma_start(xT_sb[:], xT_v[:])
            xT8_sb = constp.tile([128, KT_IN, R], F8)
            nc.sync.dma_start(xT8_sb[:], xT8_v[:])
            b1_sb = constp.tile([128, KT_BIG], F32)
            nc.sync.dma_start(b1_sb[:], b1_v[:])
            b3_sb = constp.tile([128, KT_BIG], F32)
            nc.sync.dma_start(b3_sb[:], b3_v[:])
            if with_b2:
                b2row_f = constp.tile([1, F_OUT], F32)
                nc.sync.dma_start(b2row_f[:], b2_d[None, :])
                b2row = constp.tile([1, F_OUT], BF)
                nc.vector.tensor_copy(b2row[:], b2row_f[:])
                ones_row = constp.tile([1, 128], BF)
                nc.vector.memset(ones_row[:], 1.0)
            ones_col = constp.tile([128, 1], F32)
            nc.vector.memset(ones_col[:], 1.0)
            dAcc = constp.tile([128, R], F32)
            d_row = constp.tile([1, R], F32)
            dT_sb = constp.tile([128, RT], F32)
            rd_sb = constp.tile([128, RT], F32)

            SD = F8 if fp8_d else BF
            hT_sb = hp.tile([128, KT_BIG, R], F8)   # h transposed  [F_OUT, R]
            eT_sb = ep.tile([128, KT_BIG, R], SD)   # e transposed  [N, R]

            s_in_dram = dramp.tile([R, F_OUT], SD)
            s_out_dram = dramp.tile([N, F_OUT], SD)
            d_dram = dramp.tile([R], F32)
            s_in_v = s_in_dram.rearrange("(rt p) c -> p rt c", p=128)
            s_out_v = s_out_dram.rearrange("(kt p) c -> p kt c", p=128)

            def stage_b_block(cb):
                for cb in [cb]:
                    w2_sb = w2p.tile([128, KT_IN, 512], F8, name="w2_sb")
                    nc.sync.dma_start(w2_sb[:], w2_v[:, :, ts(cb, 512)])
                    for rt in range(RT):
                        psB = psumb.tile([128, 512], F32, name="ps", tag="ps")
                        for u in range(KT_IN // 2):
                            nc.tensor.matmul(
                                psB[:],
                                xT8_sb[:, 2 * u : 2 * u + 2, ts(rt, 128)],
                                w2_sb[:, 2 * u : 2 * u + 2, :],
                                start=(u == 0),
                                stop=(not with_b2 and u == KT_IN // 2 - 1),
                                perf_mode=mybir.MatmulPerfMode.DoubleRow,
                            )
                        if with_b2:
                            # broadcast b2 over rows via K=1 matmul:
                            # psum += 1*(W2_SCALE*b2); undone by the relu scale
                            nc.tensor.matmul(
                                psB[:],
                                ones_row[:],
                                b2row[:, ts(cb, 512)],
                                start=False,
                                stop=True,
                            )
                        s_sb = sBp.tile([128, 512], SD, name="s_sb")
                        nc.scalar.activation(s_sb[:], psB[:], AFT.Relu,
                                             scale=1.0 / W2_SCALE)
                        nc.sync.dma_start(s_in_v[:, rt, ts(cb, 512)], s_sb[:])

            def stage_b():
                # s_i = relu(x_i @ W2 + b2), row-major -> DRAM bounce
                for cb in range(NB):
                    stage_b_block(cb)

            def stage_ba():
                # interleave B and A column-blocks: alternating independent
                # psum groups keep PE fed across group boundaries
                for blk in range(NB):
                    stage_b_block(blk)
                    stage_a_block(blk)

            def all_gather():
                nc.gpsimd.collective_compute(
                    "AllGather",
                    ALU.bypass,
                    replica_groups=[list(range(NCORES))],
                    ins=[s_in_dram[:]],
                    outs=[s_out_dram[:]],
                )

            def stage_a_block(fg):
                # hT = relu(x_i @ (16*W1) + 16*b1)^T = 16*h^T; fp8 DoubleRow
                w1_sb = w1p.tile([128, KT_IN, 512], F8, name="w1_sb")
                nc.sync.dma_start(w1_sb[:], w1_v[:, :, ts(fg, 512)])
                for fw in range(4):
                    ft = fg * 4 + fw
                    psA = psumb.tile([128, R], F32, name="ps", tag="ps")
                    for u in range(KT_IN // 2):
                        nc.tensor.matmul(
                            psA[:],
                            w1_sb[:, 2 * u : 2 * u + 2, ts(fw, 128)],
                            xT8_sb[:, 2 * u : 2 * u + 2, :],
                            start=(u == 0),
                            stop=(u == KT_IN // 2 - 1),
                            perf_mode=mybir.MatmulPerfMode.DoubleRow,
                        )
                    nc.scalar.activation(
                        hT_sb[:, ft, :], psA[:], AFT.Relu,
                        bias=b1_sb[:, ft : ft + 1],
                    )

            def stage_a():
                for fg in range(NB):
                    stage_a_block(fg)

            def stage_c():
                # eT = (exp(h @ W3 + b3) * adj)^T  [N, R]; dAcc accumulation
                for cb in range(NB):
                    w3_q = []
                    for q in range(KT_BIG // KQ):
                        w3_sb = w3p.tile([128, KQ, 512], F8, name="w3_sb")
                        nc.sync.dma_start(
                            w3_sb[:], w3_v[:, ts(q, KQ), ts(cb, 512)]
                        )
                        w3_q.append(w3_sb)
                    adj_sb = adjp.tile([128, 4, R], BF, name="adj_sb")
                    nc.sync.dma_start(adj_sb[:], adjT_v[:, ts(cb, 4), :])
                    for cw in range(4):
                        ct = cb * 4 + cw
                        psC = psumb.tile([128, R], F32, name="ps", tag="ps")
                        # fp8 DoubleRow: each matmul contracts a PAIR of
                        # k-tiles (middle AP dim = the 2 interleaved rows)
                        NP = KT_BIG // 2
                        for u in range(NP):
                            q, r = (2 * u) // KQ, (2 * u) % KQ
                            nc.tensor.matmul(
                                psC[:],
                                w3_q[q][:, r : r + 2, ts(cw, 128)],
                                hT_sb[:, 2 * u : 2 * u + 2, :],
                                start=(u == 0),
                                stop=(u == NP - 1),
                                perf_mode=mybir.MatmulPerfMode.DoubleRow,
                            )
                        ex_sb = expp.tile([128, R], BF, name="ex_sb")
                        nc.scalar.activation(
                            ex_sb[:], psC[:], AFT.Exp,
                            bias=b3_sb[:, ct : ct + 1],
                            scale=1.0 / (W3_SCALE * W1_SCALE),
                        )
                        nc.vector.tensor_tensor(
                            eT_sb[:, ct, :], ex_sb[:], adj_sb[:, cw, :], op=ALU.mult
                        )
                        if ct == 0:
                            nc.vector.tensor_copy(dAcc[:], eT_sb[:, ct, :])
                        else:
                            nc.vector.tensor_tensor(
                                dAcc[:], dAcc[:], eT_sb[:, ct, :], op=ALU.add
                            )

                # rowsums: reduce dAcc over partitions with a ones matmul,
                # round-trip through DRAM to relayout [1,512] -> [128,4]
                psD1 = psumd.tile([1, R], F32, name="psD1", tag="psD1")
                nc.tensor.matmul(psD1[:], ones_col[:], dAcc[:], start=True, stop=True)
                nc.scalar.copy(d_row[:], psD1[:])
                nc.sync.dma_start(d_dram.rearrange("(a r) -> a r", a=1), d_row[:])
                nc.sync.dma_start(dT_sb[:], d_dram.rearrange("(t p) -> p t", p=128))
                nc.vector.reciprocal(rd_sb[:], dT_sb[:])

            def stage_d():
                # out_i = diag(1/d) (e_i @ s)  [R, F_OUT]
                for cb in range(NB):
                    s_q = []
                    for q in range(KT_BIG // KQ):
                        sD_sb = sDp.tile([128, KQ, 512], SD, name="sD_sb")
                        nc.sync.dma_start(
                            sD_sb[:], s_out_v[:, ts(q, KQ), ts(cb, 512)]
                        )
                        s_q.append(sD_sb)
                    for rt in range(RT):
                        psE = psumb.tile([128, 512], F32, name="ps", tag="ps")
                        if fp8_d:
                            NP = KT_BIG // 2
                            for u in range(NP):
                                q, r = (2 * u) // KQ, (2 * u) % KQ
                                nc.tensor.matmul(
                                    psE[:],
                                    eT_sb[:, 2 * u : 2 * u + 2, ts(rt, 128)],
                                    s_q[q][:, r : r + 2, :],
                                    start=(u == 0),
                                    stop=(u == NP - 1),
                                    perf_mode=mybir.MatmulPerfMode.DoubleRow,
                                )
                        else:
                            for kt in range(KT_BIG):
                                nc.tensor.matmul(
                                    psE[:],
                                    eT_sb[:, kt, ts(rt, 128)],
                                    s_q[kt // KQ][:, kt % KQ, :],
                                    start=(kt == 0),
                                    stop=(kt == KT_BIG - 1),
                                )
                        ob = outp.tile([128, 512], F32, name="ob")
                        nc.vector.tensor_scalar_mul(
                            ob[:], psE[:], rd_sb[:, rt : rt + 1]
                        )
                        nc.sync.dma_start(out_v[:, rt, ts(cb, 512)], ob[:])

            if loop_reps is None:
                stage_ba()
                all_gather()
                stage_c()
                stage_d()
            else:
                stage_b()
                all_gather()
                stage_a()
                stage_c()  # so eT/rd are valid even if the loop omits stages
                with tc.For_i(0, loop_reps, 1):
                    if "b" in parts and "a" in parts:
                        stage_ba()
                    elif "b" in parts:
                        stage_b()
                    elif "a" in parts:
                        stage_a()
                    if "c" in parts:
                        stage_c()
                    if "d" in parts:
                        stage_d()

    nc.compile()
    return nc


def make_in_maps(x, adj, W1, b1, W2, b2, W3, b3):
    bf = ml_dtypes.bfloat16
    xT = np.ascontiguousarray(x.T).astype(bf)        # [F_IN, N]
    adjT = np.ascontiguousarray(adj.T).astype(bf)    # [N, N] (cols x rows)
    f8 = ml_dtypes.float8_e4m3
    w3b = np.clip(np.ascontiguousarray(W3) * W3_SCALE, -240, 240).astype(f8)
    w1b = np.clip(np.ascontiguousarray(W1) * W1_SCALE, -240, 240).astype(f8)
    w2b = np.clip(np.ascontiguousarray(W2) * W2_SCALE, -240, 240).astype(f8)
    xT8 = np.clip(xT.astype(np.float32), -240, 240).astype(f8)
    b1f = np.ascontiguousarray(b1).astype(np.float32) * np.float32(W1_SCALE)
    b2f = np.ascontiguousarray(b2).astype(np.float32) * np.float32(W2_SCALE)
    b3f = np.ascontiguousarray(b3).astype(np.float32)
    in_maps = []
    for i in range(NCORES):
        sl = slice(i * R, (i + 1) * R)
        in_maps.append(
            {
                "xT": np.ascontiguousarray(xT[:, sl]),
                "xT8": np.ascontiguousarray(xT8[:, sl]),
                "adjT": np.ascontiguousarray(adjT[:, sl]),
                "w1": w1b,
                "w2": w2b,
                "w3": w3b,
                "b1": b1f,
                "b2": b2f,
                "b3": b3f,
            }
        )
    return in_maps


def run(x, adj, W1, b1, W2, b2, W3, b3, trace=False, fp8_d=True):
    nc = build_nc(with_b2=bool(np.any(np.asarray(b2))), fp8_d=fp8_d)
    in_maps = make_in_maps(x, adj, W1, b1, W2, b2, W3, b3)
    res = run_bass_kernel_spmd(nc, in_maps, core_ids=list(range(NCORES)), trace=trace)
    out = np.concatenate([res.results[i]["out"] for i in range(NCORES)], axis=0)
    return out.astype(np.float32), res


def kernel(x, adj, W1, b1, W2, b2, W3, b3):
    args = [np.asarray(a) for a in (x, adj, W1, b1, W2, b2, W3, b3)]
    out, _ = run(*args, trace=False)
    return out

